# revision 8
# baseline (speedup 1.0000x reference)
"""DeepSeekMoE (7 routed experts top-3 + 1 shared expert) on 8 NeuronCores.

Strategy: expert-parallel with host-side dispatch (same sharding as before),
but a FUSED single-phase device kernel:
  - Cores 0-6 own one routed expert (slot A, capacity 3584 tokens) plus a
    576-token chunk of the shared expert (slot B). Core 7 runs the shared
    expert in both slots (3584 + 576). 8192 shared tokens = 7*576 + 4160.
  - Per 512-token block: GEMM1 (x@W1, x@Wg -> silu*mult -> H in SBUF, bf16)
    immediately followed by GEMM2 (H.T@W2 * scale) -> y (bf16). No DRAM
    round-trip for H; W2 stays SBUF-resident per slot; W1/Wg stream per
    block in h-slices so the PE never waits on a bulk weight load.
  - Host pre-tiles x and weights so every DMA moves long contiguous lines.
  - Host scatter-adds the per-core bf16 outputs into the full [B,S,D] f32.
"""

import threading

import numpy as np
import ml_dtypes

import concourse.bacc as bacc
import concourse.mybir as mybir
import concourse.tile as tile
from concourse.bass_utils import run_bass_kernel_spmd

BF16 = mybir.dt.bfloat16
F32 = mybir.dt.float32
NP_BF16 = ml_dtypes.bfloat16

B, S, D, H = 4, 2048, 2048, 2048
E, TOPK = 7, 3
NTOK = B * S                  # 8192 tokens
T_A, T_B = 3584, 576          # per-core slot capacities
T = T_A + T_B                 # 4160 tokens per core
NBLK = 8                      # full 512-token blocks (slot A: 7, slot B: 1)
TR = 64                       # remainder tokens (slot B)
SC_COLS = (T + 127) // 128    # 33 columns of the per-token scale tile
KT = D // 128                 # 16 contraction k-tiles for GEMM1
HT16 = H // 128               # 16 h-tiles
N_CORES = 8

TRACE = False
LAST_RESULT = None

_nc_cache = {}
_nc_lock = threading.Lock()


def _build_nc(loop_k=1):
    """Build + schedule the per-core Bass module (one NEFF, SPMD on 8 cores).

    loop_k > 1 wraps the body in a hardware For_i loop that repeats the
    (idempotent) body loop_k times — used only for on-device timing.
    """
    import contextlib

    nc = bacc.Bacc("TRN2", target_bir_lowering=False, debug=False,
                   num_devices=N_CORES)

    # host-pre-tiled inputs (all bf16):
    #   xt[b][p, k, t]   = x[b*512+t, k*128+p]          (8 full blocks)
    #   xr[p, k, t]      = x[4096+t, k*128+p]           (64-token remainder)
    #   w1 / wg [h][p, k, c] = W[k*128+p, h*128+c]      (GEMM1 stationary)
    #   w2 [k][p, d]     = W2[k*128+p, d]               (GEMM2 moving)
    xt = nc.dram_tensor("xt", [NBLK, 128, KT, 512], BF16, kind="ExternalInput")
    xr = nc.dram_tensor("xr", [128, KT, TR], BF16, kind="ExternalInput")
    w1a = nc.dram_tensor("w1a", [HT16, 128, KT, 128], BF16, kind="ExternalInput")
    wga = nc.dram_tensor("wga", [HT16, 128, KT, 128], BF16, kind="ExternalInput")
    w2a = nc.dram_tensor("w2a", [KT, 128, D], BF16, kind="ExternalInput")
    w1b = nc.dram_tensor("w1b", [HT16, 128, KT, 128], BF16, kind="ExternalInput")
    wgb = nc.dram_tensor("wgb", [HT16, 128, KT, 128], BF16, kind="ExternalInput")
    w2b = nc.dram_tensor("w2b", [KT, 128, D], BF16, kind="ExternalInput")
    sc = nc.dram_tensor("sc", [128, SC_COLS], F32, kind="ExternalInput")
    y = nc.dram_tensor("y", [T, D], BF16, kind="ExternalOutput")

    # groups of blocks sharing one W1/Wg h-slice stream (None = remainder)
    groups = [([0, 1], 0), ([2, 3], 0), ([4, 5], 0), ([6], 0),
              ([7, None], 1)]
    slot_w = [(w1a, wga, w2a), (w1b, wgb, w2b)]

    with tile.TileContext(nc) as tc:
        with tc.tile_pool(name="w2p", bufs=1) as w2pool, \
             tc.tile_pool(name="wsp", bufs=4) as wspool, \
             tc.tile_pool(name="xp", bufs=2) as xpool, \
             tc.tile_pool(name="hp", bufs=2) as hpool, \
             tc.tile_pool(name="sp", bufs=4) as spool, \
             tc.tile_pool(name="scp", bufs=1) as scpool, \
             tc.tile_pool(name="yp", bufs=4) as ypool, \
             tc.tile_pool(name="ps1", bufs=2, space="PSUM") as pspool, \
             tc.tile_pool(name="ps2", bufs=4, space="PSUM") as pspool2, \
             (tc.For_i(0, loop_k, 1) if loop_k > 1
              else contextlib.nullcontext()):
            sc_sb = scpool.tile([128, SC_COLS], F32)
            nc.sync.dma_start(sc_sb[:], sc[:, :])
            cur_slot = -1
            w2_sb = None
            for blks, slot in groups:
                if slot != cur_slot:
                    cur_slot = slot
                    w1d, wgd, w2d = slot_w[slot]
                    # W2 resident per slot; loads ride the scalar HWDGE ring
                    # so a slot-B WAR wait can't head-of-line-block the
                    # sync ring that feeds x and W1/Wg streams.
                    w2_sb = w2pool.tile([128, KT, D], BF16, tag="w2")
                    for k in range(KT):
                        nc.scalar.dma_start(w2_sb[:, k], w2d[k])
                xs = []
                for blk in blks:
                    if blk is not None:
                        x_sb = xpool.tile([128, KT, 512], BF16, tag="x")
                        nc.sync.dma_start(x_sb[:], xt[blk])
                        bw, c0 = 512, blk * 512
                    else:
                        x_sb = xpool.tile([128, KT, TR], BF16, tag="xr")
                        nc.sync.dma_start(x_sb[:], xr[:, :])
                        bw, c0 = TR, NBLK * 512
                    h_sb = hpool.tile([128, HT16, 512], BF16, tag="h")
                    xs.append((x_sb, h_sb, bw, c0))
                # ---- GEMM1: H[h,t] = silu(x@W1) * (x@Wg), bf16 in SBUF ----
                # k-interleaved across the group so consecutive matmuls
                # share the stationary weight tile (one h-slice stream per
                # group instead of per block).
                for h in range(HT16):
                    w1s = wspool.tile([128, KT, 128], BF16, tag="w1")
                    nc.sync.dma_start(w1s[:], w1d[h])
                    wgs = wspool.tile([128, KT, 128], BF16, tag="wg")
                    nc.sync.dma_start(wgs[:], wgd[h])
                    ps1s = [pspool.tile([128, 512], F32, tag="ps1",
                                        name=f"ps1_{bi}")
                            for bi in range(len(xs))]
                    for k in range(KT):
                        for bi, (x_sb, _, bw, _) in enumerate(xs):
                            nc.tensor.matmul(ps1s[bi][:, :bw], w1s[:, k],
                                             x_sb[:, k], start=(k == 0),
                                             stop=(k == KT - 1))
                    psgs = [pspool.tile([128, 512], F32, tag="psg",
                                        name=f"psg_{bi}")
                            for bi in range(len(xs))]
                    for k in range(KT):
                        for bi, (x_sb, _, bw, _) in enumerate(xs):
                            nc.tensor.matmul(psgs[bi][:, :bw], wgs[:, k],
                                             x_sb[:, k], start=(k == 0),
                                             stop=(k == KT - 1))
                    for bi, (_, h_sb, bw, _) in enumerate(xs):
                        sil = spool.tile([128, 512], BF16, tag="sil")
                        nc.scalar.activation(sil[:, :bw], ps1s[bi][:, :bw],
                                             mybir.ActivationFunctionType.Silu)
                        nc.vector.tensor_tensor(h_sb[:, h, :bw], sil[:, :bw],
                                                psgs[bi][:, :bw],
                                                mybir.AluOpType.mult)
                # ---- GEMM2: y[t,d] = (H.T @ W2) * scale[t], bf16 out ----
                # j-interleaved so all 4 d-slices share each stationary
                # H tile (4 PSUM banks accumulate in parallel per k step).
                for _, h_sb, bw, c0 in xs:
                    for i in range((bw + 127) // 128):
                        tw = min(128, bw - i * 128)
                        ts_ = slice(i * 128, i * 128 + tw)
                        psys = [pspool2.tile([128, 512], F32, tag="psy",
                                             name=f"psy_{j}")
                                for j in range(D // 512)]
                        for k in range(HT16):
                            for j in range(D // 512):
                                ds_ = slice(j * 512, (j + 1) * 512)
                                nc.tensor.matmul(psys[j][:tw],
                                                 h_sb[:, k, ts_],
                                                 w2_sb[:, k, ds_],
                                                 start=(k == 0),
                                                 stop=(k == HT16 - 1))
                        col = (c0 + i * 128) // 128
                        for j in range(D // 512):
                            ds_ = slice(j * 512, (j + 1) * 512)
                            ysb = ypool.tile([128, 512], BF16, tag="y")
                            nc.vector.tensor_scalar_mul(ysb[:tw], psys[j][:tw],
                                                        sc_sb[:tw, col:col + 1])
                            nc.scalar.dma_start(
                                y[c0 + i * 128:c0 + i * 128 + tw, ds_],
                                ysb[:tw])
    nc.compile()
    return nc


def _get_nc(loop_k=1):
    with _nc_lock:
        if loop_k not in _nc_cache:
            _nc_cache[loop_k] = _build_nc(loop_k)
        return _nc_cache[loop_k]


def benchmark(in_maps, iters=8, loop_k=1):
    """Time the NEFF execution with device-resident inputs."""
    import time as _time

    import jax
    from jax.sharding import Mesh, NamedSharding, PartitionSpec
    from jax.experimental.shard_map import shard_map

    from concourse import bass2jax, mybir as _mybir

    nc = _get_nc(loop_k)
    bass2jax.install_neuronx_cc_hook()

    partition_name = (nc.partition_id_tensor.name
                      if nc.partition_id_tensor else None)
    in_names, out_names, out_avals, zero_outs = [], [], [], []
    for alloc in nc.m.functions[0].allocations:
        if not isinstance(alloc, _mybir.MemoryLocationSet):
            continue
        name = alloc.memorylocations[0].name
        if alloc.kind == "ExternalInput":
            if name != partition_name:
                in_names.append(name)
        elif alloc.kind == "ExternalOutput":
            out_names.append(name)
            shape = tuple(alloc.tensor_shape)
            dtype = _mybir.dt.np(alloc.dtype)
            out_avals.append(jax.core.ShapedArray(shape, dtype))
            zero_outs.append(np.zeros(shape, dtype))
    n_params = len(in_names)
    all_names = in_names + out_names
    if partition_name is not None:
        all_names = all_names + [partition_name]

    def _exec_once(args, outs):
        extra = ([bass2jax.partition_id_tensor()]
                 if partition_name is not None else [])
        return bass2jax._bass_exec_p.bind(
            *args, *outs, *extra,
            out_avals=tuple(out_avals),
            in_names=tuple(all_names),
            out_names=tuple(out_names),
            lowering_input_output_aliases=(),
            sim_require_finite=True,
            sim_require_nnan=True,
            nc=nc,
        )

    def _body(*args):
        ins, outs = args[:n_params], list(args[n_params:])
        return tuple(_exec_once(ins, outs))

    n_cores = len(in_maps)
    devices = jax.devices()[:n_cores]
    mesh = Mesh(np.asarray(devices), ("core",))
    spec = PartitionSpec("core")
    sharded = jax.jit(
        shard_map(_body, mesh=mesh,
                  in_specs=(spec,) * (n_params + len(out_names)),
                  out_specs=(spec,) * len(out_names), check_rep=False),
        keep_unused=True)

    sh = NamedSharding(mesh, spec)
    dev_in = [
        jax.device_put(
            np.concatenate([np.asarray(in_maps[c][nm]) for c in range(n_cores)],
                           axis=0), sh)
        for nm in in_names
    ]
    dev_zero = [
        jax.device_put(np.zeros((n_cores * z.shape[0], *z.shape[1:]), z.dtype),
                       sh)
        for z in zero_outs
    ]
    out = sharded(*dev_in, *dev_zero)
    jax.block_until_ready(out)

    all_times = []
    for _ in range(iters):
        t0 = _time.perf_counter()
        out = sharded(*dev_in, *dev_zero)
        jax.block_until_ready(out)
        all_times.append(_time.perf_counter() - t0)
    best = min(all_times)
    benchmark.last_times = all_times

    results = [
        {nm: np.asarray(out[i]).reshape(n_cores, *out_avals[i].shape)[c]
         for i, nm in enumerate(out_names)}
        for c in range(n_cores)
    ]
    return best, results


def _softmax_f32(x):
    m = x.max(axis=-1, keepdims=True)
    e = np.exp((x - m).astype(np.float64))
    return (e / e.sum(axis=-1, keepdims=True)).astype(np.float32)


def _np_ffn(x, w1, wg, w2):
    h1 = x @ w1
    return ((h1 / (1.0 + np.exp(-h1))) * (x @ wg)) @ w2


def _tile_w1(w):
    """[D, H] -> [h, p, k, c] with w1t[h][p, k, c] = w[k*128+p, h*128+c]."""
    return np.ascontiguousarray(
        w.reshape(KT, 128, HT16, 128).transpose(2, 1, 0, 3))


def _tile_w2(w):
    """[H, D] -> [k, p, d]."""
    return np.ascontiguousarray(w.reshape(KT, 128, D))


def _tile_x(xg):
    """[T, D] bf16 -> (xt [NBLK,128,KT,512], xr [128,KT,TR])."""
    xt = np.ascontiguousarray(
        xg[:NBLK * 512].reshape(NBLK, 512, KT, 128).transpose(0, 3, 2, 1))
    xrem = np.ascontiguousarray(
        xg[NBLK * 512:].reshape(TR, KT, 128).transpose(2, 1, 0))
    return xt, xrem


def _dispatch(x, W1, Wg, W2, Ws1, Wsg, Ws2, gate_w, gate_b, biases):
    """Host-side routing + sharding. Returns (in_maps, core_idx, overflow, xf)."""
    x = np.asarray(x, dtype=np.float32)
    W1 = np.asarray(W1, dtype=np.float32)
    Wg = np.asarray(Wg, dtype=np.float32)
    W2 = np.asarray(W2, dtype=np.float32)
    Ws1 = np.asarray(Ws1, dtype=np.float32)
    Wsg = np.asarray(Wsg, dtype=np.float32)
    Ws2 = np.asarray(Ws2, dtype=np.float32)
    gate_w = np.asarray(gate_w, dtype=np.float32)
    gate_b = np.asarray(gate_b, dtype=np.float32)
    biases = np.asarray(biases, dtype=np.float32)

    xf = x.reshape(NTOK, D)

    # ---- routing (host): mirrors the reference math ----
    logits = xf @ gate_w + gate_b
    probas = _softmax_f32(logits)
    biased = probas + biases
    topk = np.argsort(-biased, axis=-1, kind="stable")[:, :TOPK]
    tp = np.take_along_axis(probas, topk, axis=-1)
    tp = tp / tp.sum(axis=-1, keepdims=True)

    # ---- dispatch ----
    xbf = xf.astype(NP_BF16)
    w1t = [_tile_w1(W1[e].astype(NP_BF16)) for e in range(E)]
    wgt = [_tile_w1(Wg[e].astype(NP_BF16)) for e in range(E)]
    w2t = [_tile_w2(W2[e].astype(NP_BF16)) for e in range(E)]
    ws1t, wsgt, ws2t = (_tile_w1(Ws1.astype(NP_BF16)),
                        _tile_w1(Wsg.astype(NP_BF16)),
                        _tile_w2(Ws2.astype(NP_BF16)))

    expert_tok = []
    expert_wt = []
    overflow = []
    for e in range(E):
        sel = (topk == e)
        rows = np.where(sel.any(axis=-1))[0]
        wts = (tp * sel).sum(axis=-1)[rows]
        if len(rows) > T_A:
            for t, w in zip(rows[T_A:], wts[T_A:]):
                overflow.append((int(t), e, float(w)))
            rows, wts = rows[:T_A], wts[:T_A]
        expert_tok.append(rows)
        expert_wt.append(wts.astype(np.float32))

    shared_chunks = [np.arange(T + T_B * i, T + T_B * (i + 1))
                     for i in range(E)]          # cores 0-6: T_B tokens each
    shared_chunks.append(np.arange(0, T))        # core 7: T tokens

    in_maps = []
    core_idx = []
    for c in range(N_CORES):
        if c < E:
            idx_a, wt_a = expert_tok[c], expert_wt[c]
            w1s, wgs, w2s = w1t[c], wgt[c], w2t[c]
            idx_b = shared_chunks[c]
        else:
            idx_a = shared_chunks[c][:T_A]
            wt_a = np.ones(T_A, np.float32)
            w1s, wgs, w2s = ws1t, wsgt, ws2t
            idx_b = shared_chunks[c][T_A:]
        n_a, n_b = len(idx_a), len(idx_b)

        xg = np.zeros((T, D), dtype=NP_BF16)
        xg[:n_a] = xbf[idx_a]
        xg[T_A:T_A + n_b] = xbf[idx_b]
        xt_c, xr_c = _tile_x(xg)

        s = np.zeros(SC_COLS * 128, np.float32)
        s[:n_a] = wt_a
        s[T_A:T_A + n_b] = 1.0
        sc_c = np.ascontiguousarray(s.reshape(SC_COLS, 128).T)

        in_maps.append({
            "xt": xt_c, "xr": xr_c, "sc": sc_c,
            "w1a": w1s, "wga": wgs, "w2a": w2s,
            "w1b": ws1t, "wgb": wsgt, "w2b": ws2t,
        })
        core_idx.append((idx_a, n_a, idx_b, n_b))

    return in_maps, core_idx, overflow, xf


def _combine(results, core_idx, overflow, xf, W1, Wg, W2):
    out = np.zeros((NTOK, D), np.float32)
    for c in range(N_CORES):
        yc = np.asarray(results[c]["y"], dtype=np.float32)
        idx_a, n_a, idx_b, n_b = core_idx[c]
        out[idx_a] += yc[:n_a]
        out[idx_b] += yc[T_A:T_A + n_b]

    for t, e, w in overflow:
        out[t] += w * _np_ffn(xf[t:t + 1], np.asarray(W1[e], np.float32),
                              np.asarray(Wg[e], np.float32),
                              np.asarray(W2[e], np.float32))[0]

    return out.reshape(B, S, D)


def kernel(x, W1, Wg, W2, Ws1, Wsg, Ws2, gate_w, gate_b, biases):
    global LAST_RESULT
    in_maps, core_idx, overflow, xf = _dispatch(
        x, W1, Wg, W2, Ws1, Wsg, Ws2, gate_w, gate_b, biases)

    nc = _get_nc()
    res = run_bass_kernel_spmd(nc, in_maps, core_ids=list(range(N_CORES)))
    LAST_RESULT = res

    return _combine(res.results, core_idx, overflow, xf, W1, Wg, W2)


# revision 10
# speedup vs baseline: 1.0177x; 1.0177x over previous
"""DeepSeekMoE (7 routed experts top-3 + 1 shared expert) on 8 NeuronCores.

Strategy: expert-parallel with host-side dispatch (same sharding as before),
but a FUSED single-phase device kernel:
  - Cores 0-6 own one routed expert (slot A, capacity 3584 tokens) plus a
    576-token chunk of the shared expert (slot B). Core 7 runs the shared
    expert in both slots (3584 + 576). 8192 shared tokens = 7*576 + 4160.
  - Per 512-token block: GEMM1 (x@W1, x@Wg -> silu*mult -> H in SBUF, bf16)
    immediately followed by GEMM2 (H.T@W2 * scale) -> y (bf16). No DRAM
    round-trip for H; W2 stays SBUF-resident per slot; W1/Wg stream per
    block in h-slices so the PE never waits on a bulk weight load.
  - Host pre-tiles x and weights so every DMA moves long contiguous lines.
  - Host scatter-adds the per-core bf16 outputs into the full [B,S,D] f32.
"""

import threading

import numpy as np
import ml_dtypes

import concourse.bacc as bacc
import concourse.mybir as mybir
import concourse.tile as tile
from concourse.bass_utils import run_bass_kernel_spmd

BF16 = mybir.dt.bfloat16
F32 = mybir.dt.float32
NP_BF16 = ml_dtypes.bfloat16

B, S, D, H = 4, 2048, 2048, 2048
E, TOPK = 7, 3
NTOK = B * S                  # 8192 tokens
T_A, T_B = 3584, 576          # per-core slot capacities
T = T_A + T_B                 # 4160 tokens per core
NBLK = 8                      # full 512-token blocks (slot A: 7, slot B: 1)
TR = 64                       # remainder tokens (slot B)
SC_COLS = (T + 127) // 128    # 33 columns of the per-token scale tile
KT = D // 128                 # 16 contraction k-tiles for GEMM1
HT16 = H // 128               # 16 h-tiles
N_CORES = 8

TRACE = False
LAST_RESULT = None

_nc_cache = {}
_nc_lock = threading.Lock()


def _build_nc(loop_k=1):
    """Build + schedule the per-core Bass module (one NEFF, SPMD on 8 cores).

    loop_k > 1 wraps the body in a hardware For_i loop that repeats the
    (idempotent) body loop_k times — used only for on-device timing.
    """
    import contextlib

    nc = bacc.Bacc("TRN2", target_bir_lowering=False, debug=False,
                   num_devices=N_CORES)

    # host-pre-tiled inputs (all bf16):
    #   xt[b][p, k, t]   = x[b*512+t, k*128+p]          (8 full blocks)
    #   xr[p, k, t]      = x[4096+t, k*128+p]           (64-token remainder)
    #   w1 / wg [h][p, k, c] = W[k*128+p, h*128+c]      (GEMM1 stationary)
    #   w2 [k][p, d]     = W2[k*128+p, d]               (GEMM2 moving)
    xt = nc.dram_tensor("xt", [NBLK, 128, KT, 512], BF16, kind="ExternalInput")
    xr = nc.dram_tensor("xr", [128, KT, TR], BF16, kind="ExternalInput")
    w1a = nc.dram_tensor("w1a", [HT16, 128, KT, 128], BF16, kind="ExternalInput")
    wga = nc.dram_tensor("wga", [HT16, 128, KT, 128], BF16, kind="ExternalInput")
    w2a = nc.dram_tensor("w2a", [KT, 128, D], BF16, kind="ExternalInput")
    w1b = nc.dram_tensor("w1b", [HT16, 128, KT, 128], BF16, kind="ExternalInput")
    wgb = nc.dram_tensor("wgb", [HT16, 128, KT, 128], BF16, kind="ExternalInput")
    w2b = nc.dram_tensor("w2b", [KT, 128, D], BF16, kind="ExternalInput")
    sc = nc.dram_tensor("sc", [128, SC_COLS], F32, kind="ExternalInput")
    y = nc.dram_tensor("y", [T, D], BF16, kind="ExternalOutput")

    # groups of blocks sharing one W1/Wg h-slice stream (None = remainder)
    groups = [([0, 1], 0), ([2, 3], 0), ([4, 5], 0), ([6], 0),
              ([7, None], 1)]
    slot_w = [(w1a, wga, w2a), (w1b, wgb, w2b)]

    with tile.TileContext(nc) as tc:
        with tc.tile_pool(name="w2p", bufs=1) as w2pool, \
             tc.tile_pool(name="wsp", bufs=4) as wspool, \
             tc.tile_pool(name="xp", bufs=2) as xpool, \
             tc.tile_pool(name="hp", bufs=2) as hpool, \
             tc.tile_pool(name="sp", bufs=4) as spool, \
             tc.tile_pool(name="scp", bufs=1) as scpool, \
             tc.tile_pool(name="yp", bufs=4) as ypool, \
             tc.tile_pool(name="ps1", bufs=3, space="PSUM") as pspool, \
             tc.tile_pool(name="ps2", bufs=2, space="PSUM") as pspool2, \
             (tc.For_i(0, loop_k, 1) if loop_k > 1
              else contextlib.nullcontext()):
            sc_sb = scpool.tile([128, SC_COLS], F32)
            nc.sync.dma_start(sc_sb[:], sc[:, :])
            cur_slot = -1
            w2_sb = None
            for blks, slot in groups:
                if slot != cur_slot:
                    cur_slot = slot
                    w1d, wgd, w2d = slot_w[slot]
                    # W2 resident per slot; loads ride the scalar HWDGE ring
                    # so a slot-B WAR wait can't head-of-line-block the
                    # sync ring that feeds x and W1/Wg streams.
                    w2_sb = w2pool.tile([128, KT, D], BF16, tag="w2")
                    for k in range(KT):
                        nc.scalar.dma_start(w2_sb[:, k], w2d[k])
                xs = []
                for blk in blks:
                    if blk is not None:
                        x_sb = xpool.tile([128, KT, 512], BF16, tag="x")
                        nc.sync.dma_start(x_sb[:], xt[blk])
                        bw, c0 = 512, blk * 512
                    else:
                        x_sb = xpool.tile([128, KT, TR], BF16, tag="xr")
                        nc.sync.dma_start(x_sb[:], xr[:, :])
                        bw, c0 = TR, NBLK * 512
                    h_sb = hpool.tile([128, HT16, 512], BF16, tag="h")
                    xs.append((x_sb, h_sb, bw, c0))
                # ---- GEMM1: H[h,t] = silu(x@W1) * (x@Wg), bf16 in SBUF ----
                # k-interleaved across the group so consecutive matmuls
                # share the stationary weight tile (one h-slice stream per
                # group instead of per block).
                for h in range(HT16):
                    w1s = wspool.tile([128, KT, 128], BF16, tag="w1")
                    nc.sync.dma_start(w1s[:], w1d[h])
                    wgs = wspool.tile([128, KT, 128], BF16, tag="wg")
                    nc.sync.dma_start(wgs[:], wgd[h])
                    ps1s = [pspool.tile([128, 512], F32, tag="ps1",
                                        name=f"ps1_{bi}")
                            for bi in range(len(xs))]
                    for k in range(KT):
                        for bi, (x_sb, _, bw, _) in enumerate(xs):
                            nc.tensor.matmul(ps1s[bi][:, :bw], w1s[:, k],
                                             x_sb[:, k], start=(k == 0),
                                             stop=(k == KT - 1))
                    psgs = [pspool.tile([128, 512], F32, tag="psg",
                                        name=f"psg_{bi}")
                            for bi in range(len(xs))]
                    for k in range(KT):
                        for bi, (x_sb, _, bw, _) in enumerate(xs):
                            nc.tensor.matmul(psgs[bi][:, :bw], wgs[:, k],
                                             x_sb[:, k], start=(k == 0),
                                             stop=(k == KT - 1))
                    for bi, (_, h_sb, bw, _) in enumerate(xs):
                        sil = spool.tile([128, 512], BF16, tag="sil")
                        nc.scalar.activation(sil[:, :bw], ps1s[bi][:, :bw],
                                             mybir.ActivationFunctionType.Silu)
                        nc.vector.tensor_tensor(h_sb[:, h, :bw], sil[:, :bw],
                                                psgs[bi][:, :bw],
                                                mybir.AluOpType.mult)
                # ---- GEMM2: y[t,d] = (H.T @ W2) * scale[t], bf16 out ----
                for _, h_sb, bw, c0 in xs:
                    for j in range(D // 512):
                        ds_ = slice(j * 512, (j + 1) * 512)
                        for i in range((bw + 127) // 128):
                            tw = min(128, bw - i * 128)
                            ts_ = slice(i * 128, i * 128 + tw)
                            psy = pspool2.tile([128, 512], F32, tag="psy")
                            for k in range(HT16):
                                nc.tensor.matmul(psy[:tw], h_sb[:, k, ts_],
                                                 w2_sb[:, k, ds_],
                                                 start=(k == 0),
                                                 stop=(k == HT16 - 1))
                            ysb = ypool.tile([128, 512], BF16, tag="y")
                            col = (c0 + i * 128) // 128
                            nc.vector.tensor_scalar_mul(ysb[:tw], psy[:tw],
                                                        sc_sb[:tw, col:col + 1])
                            nc.scalar.dma_start(
                                y[c0 + i * 128:c0 + i * 128 + tw, ds_],
                                ysb[:tw])
    nc.compile()
    return nc


def _get_nc(loop_k=1):
    with _nc_lock:
        if loop_k not in _nc_cache:
            _nc_cache[loop_k] = _build_nc(loop_k)
        return _nc_cache[loop_k]


def benchmark(in_maps, iters=8, loop_k=1):
    """Time the NEFF execution with device-resident inputs."""
    import time as _time

    import jax
    from jax.sharding import Mesh, NamedSharding, PartitionSpec
    from jax.experimental.shard_map import shard_map

    from concourse import bass2jax, mybir as _mybir

    nc = _get_nc(loop_k)
    bass2jax.install_neuronx_cc_hook()

    partition_name = (nc.partition_id_tensor.name
                      if nc.partition_id_tensor else None)
    in_names, out_names, out_avals, zero_outs = [], [], [], []
    for alloc in nc.m.functions[0].allocations:
        if not isinstance(alloc, _mybir.MemoryLocationSet):
            continue
        name = alloc.memorylocations[0].name
        if alloc.kind == "ExternalInput":
            if name != partition_name:
                in_names.append(name)
        elif alloc.kind == "ExternalOutput":
            out_names.append(name)
            shape = tuple(alloc.tensor_shape)
            dtype = _mybir.dt.np(alloc.dtype)
            out_avals.append(jax.core.ShapedArray(shape, dtype))
            zero_outs.append(np.zeros(shape, dtype))
    n_params = len(in_names)
    all_names = in_names + out_names
    if partition_name is not None:
        all_names = all_names + [partition_name]

    def _exec_once(args, outs):
        extra = ([bass2jax.partition_id_tensor()]
                 if partition_name is not None else [])
        return bass2jax._bass_exec_p.bind(
            *args, *outs, *extra,
            out_avals=tuple(out_avals),
            in_names=tuple(all_names),
            out_names=tuple(out_names),
            lowering_input_output_aliases=(),
            sim_require_finite=True,
            sim_require_nnan=True,
            nc=nc,
        )

    def _body(*args):
        ins, outs = args[:n_params], list(args[n_params:])
        return tuple(_exec_once(ins, outs))

    n_cores = len(in_maps)
    devices = jax.devices()[:n_cores]
    mesh = Mesh(np.asarray(devices), ("core",))
    spec = PartitionSpec("core")
    sharded = jax.jit(
        shard_map(_body, mesh=mesh,
                  in_specs=(spec,) * (n_params + len(out_names)),
                  out_specs=(spec,) * len(out_names), check_rep=False),
        keep_unused=True)

    sh = NamedSharding(mesh, spec)
    dev_in = [
        jax.device_put(
            np.concatenate([np.asarray(in_maps[c][nm]) for c in range(n_cores)],
                           axis=0), sh)
        for nm in in_names
    ]
    dev_zero = [
        jax.device_put(np.zeros((n_cores * z.shape[0], *z.shape[1:]), z.dtype),
                       sh)
        for z in zero_outs
    ]
    out = sharded(*dev_in, *dev_zero)
    jax.block_until_ready(out)

    all_times = []
    for _ in range(iters):
        t0 = _time.perf_counter()
        out = sharded(*dev_in, *dev_zero)
        jax.block_until_ready(out)
        all_times.append(_time.perf_counter() - t0)
    best = min(all_times)
    benchmark.last_times = all_times

    results = [
        {nm: np.asarray(out[i]).reshape(n_cores, *out_avals[i].shape)[c]
         for i, nm in enumerate(out_names)}
        for c in range(n_cores)
    ]
    return best, results


def _softmax_f32(x):
    m = x.max(axis=-1, keepdims=True)
    e = np.exp((x - m).astype(np.float64))
    return (e / e.sum(axis=-1, keepdims=True)).astype(np.float32)


def _np_ffn(x, w1, wg, w2):
    h1 = x @ w1
    return ((h1 / (1.0 + np.exp(-h1))) * (x @ wg)) @ w2


def _tile_w1(w):
    """[D, H] -> [h, p, k, c] with w1t[h][p, k, c] = w[k*128+p, h*128+c]."""
    return np.ascontiguousarray(
        w.reshape(KT, 128, HT16, 128).transpose(2, 1, 0, 3))


def _tile_w2(w):
    """[H, D] -> [k, p, d]."""
    return np.ascontiguousarray(w.reshape(KT, 128, D))


def _tile_x(xg):
    """[T, D] bf16 -> (xt [NBLK,128,KT,512], xr [128,KT,TR])."""
    xt = np.ascontiguousarray(
        xg[:NBLK * 512].reshape(NBLK, 512, KT, 128).transpose(0, 3, 2, 1))
    xrem = np.ascontiguousarray(
        xg[NBLK * 512:].reshape(TR, KT, 128).transpose(2, 1, 0))
    return xt, xrem


def _dispatch(x, W1, Wg, W2, Ws1, Wsg, Ws2, gate_w, gate_b, biases):
    """Host-side routing + sharding. Returns (in_maps, core_idx, overflow, xf)."""
    x = np.asarray(x, dtype=np.float32)
    W1 = np.asarray(W1, dtype=np.float32)
    Wg = np.asarray(Wg, dtype=np.float32)
    W2 = np.asarray(W2, dtype=np.float32)
    Ws1 = np.asarray(Ws1, dtype=np.float32)
    Wsg = np.asarray(Wsg, dtype=np.float32)
    Ws2 = np.asarray(Ws2, dtype=np.float32)
    gate_w = np.asarray(gate_w, dtype=np.float32)
    gate_b = np.asarray(gate_b, dtype=np.float32)
    biases = np.asarray(biases, dtype=np.float32)

    xf = x.reshape(NTOK, D)

    # ---- routing (host): mirrors the reference math ----
    logits = xf @ gate_w + gate_b
    probas = _softmax_f32(logits)
    biased = probas + biases
    topk = np.argsort(-biased, axis=-1, kind="stable")[:, :TOPK]
    tp = np.take_along_axis(probas, topk, axis=-1)
    tp = tp / tp.sum(axis=-1, keepdims=True)

    # ---- dispatch ----
    xbf = xf.astype(NP_BF16)
    w1t = [_tile_w1(W1[e].astype(NP_BF16)) for e in range(E)]
    wgt = [_tile_w1(Wg[e].astype(NP_BF16)) for e in range(E)]
    w2t = [_tile_w2(W2[e].astype(NP_BF16)) for e in range(E)]
    ws1t, wsgt, ws2t = (_tile_w1(Ws1.astype(NP_BF16)),
                        _tile_w1(Wsg.astype(NP_BF16)),
                        _tile_w2(Ws2.astype(NP_BF16)))

    expert_tok = []
    expert_wt = []
    overflow = []
    for e in range(E):
        sel = (topk == e)
        rows = np.where(sel.any(axis=-1))[0]
        wts = (tp * sel).sum(axis=-1)[rows]
        if len(rows) > T_A:
            for t, w in zip(rows[T_A:], wts[T_A:]):
                overflow.append((int(t), e, float(w)))
            rows, wts = rows[:T_A], wts[:T_A]
        expert_tok.append(rows)
        expert_wt.append(wts.astype(np.float32))

    shared_chunks = [np.arange(T + T_B * i, T + T_B * (i + 1))
                     for i in range(E)]          # cores 0-6: T_B tokens each
    shared_chunks.append(np.arange(0, T))        # core 7: T tokens

    in_maps = []
    core_idx = []
    for c in range(N_CORES):
        if c < E:
            idx_a, wt_a = expert_tok[c], expert_wt[c]
            w1s, wgs, w2s = w1t[c], wgt[c], w2t[c]
            idx_b = shared_chunks[c]
        else:
            idx_a = shared_chunks[c][:T_A]
            wt_a = np.ones(T_A, np.float32)
            w1s, wgs, w2s = ws1t, wsgt, ws2t
            idx_b = shared_chunks[c][T_A:]
        n_a, n_b = len(idx_a), len(idx_b)

        xg = np.zeros((T, D), dtype=NP_BF16)
        xg[:n_a] = xbf[idx_a]
        xg[T_A:T_A + n_b] = xbf[idx_b]
        xt_c, xr_c = _tile_x(xg)

        s = np.zeros(SC_COLS * 128, np.float32)
        s[:n_a] = wt_a
        s[T_A:T_A + n_b] = 1.0
        sc_c = np.ascontiguousarray(s.reshape(SC_COLS, 128).T)

        in_maps.append({
            "xt": xt_c, "xr": xr_c, "sc": sc_c,
            "w1a": w1s, "wga": wgs, "w2a": w2s,
            "w1b": ws1t, "wgb": wsgt, "w2b": ws2t,
        })
        core_idx.append((idx_a, n_a, idx_b, n_b))

    return in_maps, core_idx, overflow, xf


def _combine(results, core_idx, overflow, xf, W1, Wg, W2):
    out = np.zeros((NTOK, D), np.float32)
    for c in range(N_CORES):
        yc = np.asarray(results[c]["y"], dtype=np.float32)
        idx_a, n_a, idx_b, n_b = core_idx[c]
        out[idx_a] += yc[:n_a]
        out[idx_b] += yc[T_A:T_A + n_b]

    for t, e, w in overflow:
        out[t] += w * _np_ffn(xf[t:t + 1], np.asarray(W1[e], np.float32),
                              np.asarray(Wg[e], np.float32),
                              np.asarray(W2[e], np.float32))[0]

    return out.reshape(B, S, D)


def kernel(x, W1, Wg, W2, Ws1, Wsg, Ws2, gate_w, gate_b, biases):
    global LAST_RESULT
    in_maps, core_idx, overflow, xf = _dispatch(
        x, W1, Wg, W2, Ws1, Wsg, Ws2, gate_w, gate_b, biases)

    nc = _get_nc()
    res = run_bass_kernel_spmd(nc, in_maps, core_ids=list(range(N_CORES)))
    LAST_RESULT = res

    return _combine(res.results, core_idx, overflow, xf, W1, Wg, W2)
